# revision 12
# baseline (speedup 1.0000x reference)
"""GCNConv layer on 8 Trainium2 NeuronCores (Bass/Tile).

Strategy (graph/data parallel, edges partitioned by destination):
  out = relu( D^-1/2 (A+I) D^-1/2 (x W) + b ) + x
      = relu( (dinv_d * (sum_{e->d} dinv_s x_s + dinv_d x_d)) @ W + b ) + x
(using linearity: the W matmul is applied after aggregation).

Each core owns N/8 destination nodes. Per core:
  - source nodes are split into 4 chunks of N/4 rows so gather indices fit
    int16 (dma_gather requirement)
  - per (chunk c, dst-half h): destinations ordered by in-degree from chunk c
    (descending), so the k-th incoming edge of every dst forms a *prefix* of
    the ordering (ELL layout).  Pass k = one dma_gather of the k-th edges'
    source rows + DVE multiply by dinv_src + DVE accumulate.
  - gathers are spread round-robin over the 4 SWDGE queues: each queue's
    descriptor generation runs on its own Q7 core pair, ~4x faster than one.
  - per-phase accumulator is scaled by dinv_d (rank order) and written
    *densely* to a per-chunk HBM table; the final phase re-gathers the four
    permuted contributions per 512-row group (no scatter-add, no RMW chain).
  - final: sum 4 gathered tables + resident self-loop term dinv_d^2 x_d,
    transpose via PE, matmul with W, fused bias+relu on ACT, transpose back,
    add residual x, store.

Edge sets are padded with weight-0 fake edges so all 8 cores run the exact
same static program (SPMD) with per-core data only.
"""

import sys
import types

sys.path.insert(0, "/opt/trn_rl_repo")

import numpy as np

DIM = 64
N_CORES = 8
N_CHUNKS = 4
N_HALVES = 2
N_QUEUES = 4
P = 128
GB = 8  # dst blocks per final-phase group


def _install_ntff_hook():
    """run_bass_kernel_spmd(trace=True) needs antenv.axon_hooks; the image
    lacks it - install an equivalent backed by libaxon_pjrt.so."""
    if "antenv.axon_hooks" in sys.modules:
        return
    try:
        sys.path.insert(0, "/root/.axon_site")
        from trn_agent_boot.trn_boot import _ntff_profile_via_ctypes

        hook = _ntff_profile_via_ctypes("/opt/axon/libaxon_pjrt.so")
    except Exception:
        hook = None
    mod = types.ModuleType("antenv.axon_hooks")
    mod.get_axon_ntff_profile_hook = lambda: hook
    mod.set_axon_ntff_profile_hook = lambda h: None
    sys.modules["antenv.axon_hooks"] = mod


class Plan:
    """Static (core-independent) program structure."""

    def __init__(self, n_nodes, n_cores, n_chunks, n_halves):
        assert n_nodes % n_cores == 0
        assert n_nodes % n_chunks == 0
        self.N = n_nodes
        self.n_cores = n_cores
        self.n_chunks = n_chunks
        self.n_halves = n_halves
        self.SHARD = n_nodes // n_cores          # dst rows per core
        self.CH = n_nodes // n_chunks            # src rows per chunk
        assert self.CH <= 32767, "chunk must fit int16 index"
        assert self.SHARD % n_halves == 0
        self.HALF = self.SHARD // n_halves       # dst rows per phase
        self.SHB = -(-self.SHARD // P)           # shard blocks (ceil)
        self.ACCB = -(-self.HALF // P)           # accumulator blocks
        self.ACC_SLOTS = self.ACCB * P
        assert self.ACC_SLOTS % 16 == 0
        self.n_phases = n_chunks * n_halves
        self.n_groups = -(-self.SHB // GB)
        self.pass_sizes = None   # [phase][k] -> padded slot count (all cores)
        self.g16_off = None      # [phase][k] -> col offset into gidx blob
        self.g128_off = None     # [phase][k] -> col offset into gwgt blob
        self.GCOLS = 0
        self.WCOLS = 0
        # merge-gather idx blob layout: per (group, chunk) a [128, gsz/16]
        self.group_sizes = [
            min(GB, self.SHB - g * GB) * P for g in range(self.n_groups)
        ]
        self.MG_COLS = sum(s // 16 for s in self.group_sizes) * n_chunks

    def mg_off(self, g, c):
        o = 0
        for gg in range(g):
            o += (self.group_sizes[gg] // 16) * self.n_chunks
        return o + (self.group_sizes[g] // 16) * c


def _rep16(vals_i16, n):
    """[n] int -> [128, n//16] wrapped-in-16-partitions, replicated 8x."""
    a = np.asarray(vals_i16, dtype=np.int16).reshape(n // 16, 16).T  # [16, n/16]
    return np.tile(a, (8, 1))


def preprocess(x, edge_index, W, b):
    """Host-side sharding: build the static plan + per-core input maps."""
    x = np.ascontiguousarray(np.asarray(x, dtype=np.float32))
    N = x.shape[0]
    plan = Plan(N, N_CORES, N_CHUNKS, N_HALVES)
    src = np.asarray(edge_index[0], dtype=np.int64)
    dst = np.asarray(edge_index[1], dtype=np.int64)
    deg = np.bincount(dst, minlength=N).astype(np.float64) + 1.0
    dinv = (1.0 / np.sqrt(deg)).astype(np.float32)

    SHARD, CH, HALF = plan.SHARD, plan.CH, plan.HALF
    NPH = plan.n_phases

    core_of = dst // SHARD
    per_core = []  # [core][phase] -> dict(passes=[src arrays], rank)
    for i in range(N_CORES):
        m = core_of == i
        s_i = src[m]
        d_i = dst[m] - i * SHARD
        c_i = s_i // CH
        h_i = d_i // HALF
        phases = []
        for c in range(N_CHUNKS):
            for h in range(N_HALVES):
                mm = (c_i == c) & (h_i == h)
                s = s_i[mm]
                d = d_i[mm] - h * HALF
                deg_ch = np.bincount(d, minlength=HALF)
                order = np.argsort(-deg_ch, kind="stable")  # rank -> dst slot
                rank = np.empty(HALF, dtype=np.int64)
                rank[order] = np.arange(HALF)
                perm = np.argsort(rank[d], kind="stable")
                s_sorted = s[perm]
                counts = deg_ch[order]                  # per rank, descending
                cum = np.concatenate([[0], np.cumsum(counts)])
                K = int(counts[0]) if len(s) else 0
                passes = []
                for k in range(K):
                    L = int(np.searchsorted(-counts, -k, side="left"))
                    pos = cum[:L] + k
                    passes.append(s_sorted[pos])
                phases.append({"passes": passes, "rank": rank})
        per_core.append(phases)

    # static pass structure: max over cores, pad to 128
    pass_sizes = []
    for ph in range(NPH):
        K = max(len(per_core[i][ph]["passes"]) for i in range(N_CORES))
        sizes = []
        for k in range(K):
            L = max(
                len(per_core[i][ph]["passes"][k])
                if k < len(per_core[i][ph]["passes"])
                else 0
                for i in range(N_CORES)
            )
            sizes.append(-(-L // P) * P)
        pass_sizes.append(sizes)
    plan.pass_sizes = pass_sizes

    g16_off, g128_off = [], []
    o16 = o128 = 0
    for ph in range(NPH):
        offs16, offs128 = [], []
        for n in pass_sizes[ph]:
            offs16.append(o16)
            offs128.append(o128)
            o16 += n // 16
            o128 += n // P
        g16_off.append(offs16)
        g128_off.append(offs128)
    plan.g16_off, plan.g128_off = g16_off, g128_off
    plan.GCOLS = max(o16, 16)
    plan.WCOLS = max(o128, 1)

    # per-core blobs
    in_maps = []
    W = np.ascontiguousarray(np.asarray(W, dtype=np.float32))
    b = np.ascontiguousarray(np.asarray(b, dtype=np.float32).reshape(DIM, 1))
    for i in range(N_CORES):
        gidx = np.zeros((P, plan.GCOLS), dtype=np.int16)
        for ph in range(NPH):
            c = ph // N_HALVES
            pdata = per_core[i][ph]
            for k, n in enumerate(pass_sizes[ph]):
                s_pass = (
                    pdata["passes"][k]
                    if k < len(pdata["passes"])
                    else np.empty(0, np.int64)
                )
                L = len(s_pass)
                iv = np.full(n, CH, dtype=np.int16)  # pad -> zero row
                iv[:L] = (s_pass - c * CH).astype(np.int16)
                gidx[:, g16_off[ph][k] : g16_off[ph][k] + n // 16] = _rep16(iv, n)
        # dinv of own dst rows, in rank order per (c,h): used to pre-scale acc
        dinvr = np.zeros((P, NPH * plan.ACCB), dtype=np.float32)
        for ph in range(NPH):
            c, h = ph // N_HALVES, ph % N_HALVES
            rank = per_core[i][ph]["rank"]
            dv = np.zeros(plan.ACC_SLOTS, dtype=np.float32)
            # rank r -> dst slot order[r]; dinv value of that dst
            order = np.empty(HALF, dtype=np.int64)
            order[rank] = np.arange(HALF)
            dv[:HALF] = dinv[i * SHARD + h * HALF + order]
            dinvr[:, ph * plan.ACCB : (ph + 1) * plan.ACCB] = dv.reshape(
                plan.ACCB, P
            ).T
        # merge-gather indices: for final group g, chunk c: row d -> h*ACC_SLOTS+rank
        mgidx = np.zeros((P, plan.MG_COLS), dtype=np.int16)
        for g in range(plan.n_groups):
            gsz = plan.group_sizes[g]
            d = np.arange(g * GB * P, g * GB * P + gsz)
            dc = np.clip(d, 0, SHARD - 1)
            hh = dc // HALF
            for c in range(N_CHUNKS):
                ph = c * N_HALVES
                iv = np.zeros(gsz, dtype=np.int16)
                for h in range(N_HALVES):
                    mh = hh == h
                    rank = per_core[i][ph + h]["rank"]
                    iv[mh] = (h * plan.ACC_SLOTS + rank[dc[mh] - h * HALF]).astype(
                        np.int16
                    )
                iv[d >= SHARD] = 0
                o = plan.mg_off(g, c)
                mgidx[:, o : o + gsz // 16] = _rep16(iv, gsz)
        # dinv over all nodes in chunk-block layout for the xs-table build
        CB = -(-CH // P)
        dva = np.zeros((N_CHUNKS * CB * P,), dtype=np.float32)
        for c in range(N_CHUNKS):
            dva[c * CB * P : c * CB * P + CH] = dinv[c * CH : (c + 1) * CH]
        dinvch = np.ascontiguousarray(
            dva.reshape(N_CHUNKS * CB, P).T
        )  # [128, N_CHUNKS*CB]
        # self-loop scale dinv^2 in node order, [p,b] = val[b*128+p]
        dv = np.zeros((plan.SHB * P,), dtype=np.float32)
        dv[:SHARD] = dinv[i * SHARD : (i + 1) * SHARD] ** 2
        dinvsq = np.ascontiguousarray(dv.reshape(plan.SHB, P).T)
        xsh = np.ascontiguousarray(x[i * SHARD : (i + 1) * SHARD])
        in_maps.append(
            {
                "x": x,
                "xsh": xsh,
                "w": W,
                "bias": b,
                "dinvsq": dinvsq,
                "dinvr": dinvr,
                "dinvch": dinvch,
                "gidx": gidx,
                "mgidx": mgidx,
            }
        )
    return plan, in_maps


_QPATCHED = [False]


def _patch_queue_aware_dma_lanes():
    """Tile assigns DMA-completion sem lanes (DMASW0-7) round-robin in
    scheduled order, ignoring queue_num.  Two SWDGE queues sharing a lane can
    complete out of order and release waiters early.  Partition the 8 lanes
    so queue q owns lanes {2q, 2q+1}."""
    if _QPATCHED[0]:
        return
    _QPATCHED[0] = True
    from concourse import tile_sem_assignment as tsa
    from concourse import bass_isa, mybir

    orig = tsa.TileClockTick._assign_tick

    def qaware(self, inst):
        if (
            isinstance(inst, tsa.DMAInst)
            and inst.engine == mybir.EngineType.Pool
            and not isinstance(inst, bass_isa.UserSyncedRemoteDMADescs)
        ):
            qn = getattr(inst, "queue_num", 0) or 0
            tog = getattr(self, "_q_toggle", None)
            if tog is None:
                tog = self._q_toggle = {}
            t = tog.get(qn, 0)
            tog[qn] = t ^ 1
            self.next_sw_dma_idx = 2 * qn + t
        return orig(self, inst)

    tsa.TileClockTick._assign_tick = qaware


def build_program(plan):
    from concourse import bacc, mybir
    import concourse.tile as tile
    from concourse.masks import make_identity
    from concourse.tile import add_dep_helper

    _patch_queue_aware_dma_lanes()

    N = plan.N
    SHARD, CH, HALF = plan.SHARD, plan.CH, plan.HALF
    SHB, ACCB = plan.SHB, plan.ACCB
    NPH = plan.n_phases
    FB = SHARD // P              # full shard blocks
    REM = SHARD - FB * P         # partial block rows
    f32 = mybir.dt.float32
    i16 = mybir.dt.int16
    mult = mybir.AluOpType.mult
    add = mybir.AluOpType.add

    nc = bacc.Bacc("TRN2", target_bir_lowering=False, num_swdge_queues=N_QUEUES)
    x_d = nc.dram_tensor("x", [N, DIM], f32, kind="ExternalInput")
    xsh_d = nc.dram_tensor("xsh", [SHARD, DIM], f32, kind="ExternalInput")
    w_d = nc.dram_tensor("w", [DIM, DIM], f32, kind="ExternalInput")
    b_d = nc.dram_tensor("bias", [DIM, 1], f32, kind="ExternalInput")
    dinvsq_d = nc.dram_tensor("dinvsq", [P, SHB], f32, kind="ExternalInput")
    dinvr_d = nc.dram_tensor("dinvr", [P, NPH * ACCB], f32, kind="ExternalInput")
    gidx_d = nc.dram_tensor("gidx", [P, plan.GCOLS], i16, kind="ExternalInput")
    CB = -(-CH // P)
    dinvch_d = nc.dram_tensor("dinvch", [P, N_CHUNKS * CB], f32, kind="ExternalInput")
    mgidx_d = nc.dram_tensor("mgidx", [P, plan.MG_COLS], i16, kind="ExternalInput")
    # pre-scaled gather tables xs = dinv_s * x_s, one per chunk, + zero row at CH
    xst = [
        nc.dram_tensor(f"xst{c}", [CB * P + P, DIM], f32) for c in range(N_CHUNKS)
    ]
    # per-chunk permuted aggregate tables (both halves stacked)
    accd = [
        nc.dram_tensor(f"accd{c}", [N_HALVES * plan.ACC_SLOTS, DIM], f32)
        for c in range(N_CHUNKS)
    ]
    out_d = nc.dram_tensor("out", [SHARD, DIM], f32, kind="ExternalOutput")

    max_nblk = max((max(s) for s in plan.pass_sizes if s), default=P) // P
    qctr = [0]

    def next_q():
        q = qctr[0] % N_QUEUES
        qctr[0] += 1
        return q

    with tile.TileContext(nc) as tc:
        with (
            tc.tile_pool(name="const", bufs=1) as constp,
            tc.tile_pool(name="io", bufs=3) as iop,
            tc.tile_pool(name="gbuf", bufs=6) as gbufp,
            tc.tile_pool(name="accp", bufs=2) as accp,
            tc.tile_pool(name="fin", bufs=2) as finp,
            tc.tile_pool(name="psum", bufs=2, space="PSUM") as psump,
            tc.tile_pool(name="psum1", bufs=1, space="PSUM") as psum1p,
        ):
            ident = constp.tile([P, P], f32)
            make_identity(nc, ident[:])
            w_t = constp.tile([DIM, DIM], f32)
            nc.sync.dma_start(out=w_t[:], in_=w_d[:])
            b_t = constp.tile([DIM, 1], f32)
            nc.sync.dma_start(out=b_t[:], in_=b_d[:])
            dinvsq_t = constp.tile([P, SHB], f32)
            nc.sync.dma_start(out=dinvsq_t[:], in_=dinvsq_d[:])
            dinvr_t = constp.tile([P, NPH * ACCB], f32)
            nc.sync.dma_start(out=dinvr_t[:], in_=dinvr_d[:])
            dinvch_t = constp.tile([P, N_CHUNKS * CB], f32)
            nc.sync.dma_start(out=dinvch_t[:], in_=dinvch_d[:])
            mgidx_t = constp.tile([P, plan.MG_COLS], i16)
            nc.sync.dma_start(out=mgidx_t[:], in_=mgidx_d[:])

            # x shard resident: [128, SHB*64], row b*128+p -> [p, b*64:(b+1)*64]
            xs_t = constp.tile([P, SHB * DIM], f32)
            if REM:
                nc.vector.memset(xs_t[:, FB * DIM :], 0.0)
            nc.sync.dma_start(
                out=xs_t[:, : FB * DIM].rearrange("p (bb d) -> p bb d", d=DIM),
                in_=xsh_d[0 : FB * P, :].rearrange("(bb p) d -> p bb d", p=P),
            )
            if REM:
                nc.sync.dma_start(
                    out=xs_t[:REM, FB * DIM :],
                    in_=xsh_d[FB * P : SHARD, :],
                )


            # build pre-scaled gather tables xs_c = dinv_s * x_s (dense, HWDGE)
    
            zrow = constp.tile([P, DIM], f32)
            nc.vector.memset(zrow[:], 0.0)
            xst_writes = [[] for _ in range(N_CHUNKS)]
            TB = 24  # blocks per build tile
            FBc = CH // P
            REMc = CH - FBc * P
            for c in range(N_CHUNKS):
                for t0 in range(0, FBc, TB):
                    nb = min(TB, FBc - t0)
                    bt = iop.tile([P, TB * DIM], f32, tag="bld")
                    nc.sync.dma_start(
                        out=bt[:, : nb * DIM].rearrange("p (bb d) -> p bb d", d=DIM),
                        in_=x_d[
                            c * CH + t0 * P : c * CH + (t0 + nb) * P, :
                        ].rearrange("(bb p) d -> p bb d", p=P),
                    )
                    nc.vector.tensor_tensor(
                        out=bt[:, : nb * DIM].rearrange("p (bb d) -> p bb d", d=DIM),
                        in0=bt[:, : nb * DIM].rearrange("p (bb d) -> p bb d", d=DIM),
                        in1=dinvch_t[:, c * CB + t0 : c * CB + t0 + nb].to_broadcast(
                            [P, nb, DIM]
                        ),
                        op=mult,
                    )
                    wi = nc.sync.dma_start(
                        out=xst[c][t0 * P : (t0 + nb) * P, :].rearrange(
                            "(bb p) d -> p bb d", p=P
                        ),
                        in_=bt[:, : nb * DIM].rearrange("p (bb d) -> p bb d", d=DIM),
                    )
                    xst_writes[c].append(wi)
                if REMc:
                    bt = iop.tile([P, TB * DIM], f32, tag="bld")
                    nc.vector.memset(bt[:, :DIM], 0.0)
                    nc.sync.dma_start(
                        out=bt[:REMc, :DIM],
                        in_=x_d[c * CH + FBc * P : (c + 1) * CH, :],
                    )
                    nc.vector.tensor_tensor(
                        out=bt[:, :DIM].rearrange("p (bb d) -> p bb d", d=DIM),
                        in0=bt[:, :DIM].rearrange("p (bb d) -> p bb d", d=DIM),
                        in1=dinvch_t[:, c * CB + FBc : c * CB + FBc + 1].to_broadcast(
                            [P, 1, DIM]
                        ),
                        op=mult,
                    )
                    wi = nc.sync.dma_start(
                        out=xst[c][FBc * P : (FBc + 1) * P, :].rearrange(
                            "(bb p) d -> p bb d", p=P
                        ),
                        in_=bt[:, :DIM].rearrange("p (bb d) -> p bb d", d=DIM),
                    )
                    xst_writes[c].append(wi)
                # zero row at index CH (plus padding rows)
                wi = nc.sync.dma_start(
                    out=xst[c][CB * P : CB * P + P, :], in_=zrow[:]
                )
                xst_writes[c].append(wi)

            # aggregation phases
            accd_writes = []
            for ph in range(NPH):
                c, h = ph // N_HALVES, ph % N_HALVES
                sizes = plan.pass_sizes[ph]
                if sizes:
                    gcols = plan.g16_off[ph][-1] + sizes[-1] // 16 - plan.g16_off[ph][0]
                    gidx_t = iop.tile([P, gcols], i16, tag="gidx")
                    nc.sync.dma_start(
                        out=gidx_t[:],
                        in_=gidx_d[
                            :, plan.g16_off[ph][0] : plan.g16_off[ph][0] + gcols
                        ],
                    )
                acc_t = accp.tile([P, ACCB * DIM], f32, tag="acc")
                n0blk = (sizes[0] // P) if sizes else 0
                if n0blk < ACCB:
                    nc.vector.memset(acc_t[:, n0blk * DIM :], 0.0)
                for k, n in enumerate(sizes):
                    nblk = n // P
                    o16 = plan.g16_off[ph][k] - plan.g16_off[ph][0]
                    if k == 0:
                        gout = acc_t[:, : nblk * DIM]
                    else:
                        buf = gbufp.tile([P, max_nblk * DIM], f32, tag="gb")
                        gout = buf[:, : nblk * DIM]
                    ginst = nc.gpsimd.dma_gather(
                        out_ap=gout.rearrange("p (j d) -> p j d", d=DIM),
                        in_ap=xst[c][:, :],
                        idxs_ap=gidx_t[:, o16 : o16 + n // 16],
                        num_idxs=n,
                        num_idxs_reg=n,
                        elem_size=DIM,
                        single_packet=False,
                        queue_num=next_q(),
                    )
                    for wi in xst_writes[c]:
                        add_dep_helper(ginst.ins, wi.ins, reason="xs table before gather")
                    if k > 0:
                        nc.vector.tensor_tensor(
                            out=acc_t[:, : nblk * DIM],
                            in0=acc_t[:, : nblk * DIM],
                            in1=buf[:, : nblk * DIM],
                            op=add,
                        )
                # pre-scale by dinv_d (rank order) and write densely to HBM
                nc.vector.tensor_tensor(
                    out=acc_t[:].rearrange("p (j d) -> p j d", d=DIM),
                    in0=acc_t[:].rearrange("p (j d) -> p j d", d=DIM),
                    in1=dinvr_t[:, ph * ACCB : (ph + 1) * ACCB].to_broadcast(
                        [P, ACCB, DIM]
                    ),
                    op=mult,
                )
                winst = nc.sync.dma_start(
                    out=accd[c][
                        h * plan.ACC_SLOTS : (h + 1) * plan.ACC_SLOTS, :
                    ].rearrange("(j p) d -> p j d", p=P),
                    in_=acc_t[:].rearrange("p (j d) -> p j d", d=DIM),
                )
                accd_writes.append((c, winst))

            # final: out = relu((sum_c perm_c(accd_c) + self) @ W + b) + x
            for g in range(plan.n_groups):
                gsz = plan.group_sizes[g]
                blks = gsz // P
                mg = []
                for c in range(N_CHUNKS):
                    mb = finp.tile([P, GB * DIM], f32, tag=f"mg{c}")
                    o = plan.mg_off(g, c)
                    ginst = nc.gpsimd.dma_gather(
                        out_ap=mb[:, : blks * DIM].rearrange(
                            "p (j d) -> p j d", d=DIM
                        ),
                        in_ap=accd[c][:, :],
                        idxs_ap=mgidx_t[:, o : o + gsz // 16],
                        num_idxs=gsz,
                        num_idxs_reg=gsz,
                        elem_size=DIM,
                        single_packet=False,
                        queue_num=next_q(),
                    )
                    for cc, wi in accd_writes:
                        if cc == c:
                            add_dep_helper(
                                ginst.ins, wi.ins, reason="accd write before merge"
                            )
                    mg.append(mb)
                ag = finp.tile([P, GB * DIM], f32, tag="ag")
                # self-loop term dinv_d^2 * x_d for this group
                nc.vector.tensor_tensor(
                    out=ag[:, : blks * DIM].rearrange("p (bb d) -> p bb d", d=DIM),
                    in0=xs_t[:, g * GB * DIM : (g * GB + blks) * DIM].rearrange(
                        "p (bb d) -> p bb d", d=DIM
                    ),
                    in1=dinvsq_t[:, g * GB : g * GB + blks].to_broadcast(
                        [P, blks, DIM]
                    ),
                    op=mult,
                )
                for c in range(N_CHUNKS):
                    nc.vector.tensor_tensor(
                        out=ag[:, : blks * DIM],
                        in0=ag[:, : blks * DIM],
                        in1=mg[c][:, : blks * DIM],
                        op=add,
                    )
                pt = psum1p.tile([DIM, GB * P], f32, tag="pt")
                for bb in range(blks):
                    nc.tensor.transpose(
                        out=pt[:, bb * P : (bb + 1) * P],
                        in_=ag[:, bb * DIM : (bb + 1) * DIM],
                        identity=ident[:],
                    )
                at = finp.tile([DIM, GB * P], f32, tag="at")
                nc.vector.tensor_copy(out=at[:, : blks * P], in_=pt[:, : blks * P])
                pz = psum1p.tile([DIM, GB * P], f32, tag="pz")
                for mo in range(0, blks * P, 512):
                    mw = min(512, blks * P - mo)
                    nc.tensor.matmul(
                        out=pz[:, mo : mo + mw],
                        lhsT=w_t[:],
                        rhs=at[:, mo : mo + mw],
                        start=True,
                        stop=True,
                    )
                zr = finp.tile([DIM, GB * P], f32, tag="zr")
                nc.scalar.activation(
                    out=zr[:, : blks * P],
                    in_=pz[:, : blks * P],
                    func=mybir.ActivationFunctionType.Relu,
                    bias=b_t[:],
                )
                po = psump.tile([P, GB * DIM], f32, tag="po")
                for bb in range(blks):
                    nc.tensor.transpose(
                        out=po[:, bb * DIM : (bb + 1) * DIM],
                        in_=zr[:, bb * P : (bb + 1) * P],
                        identity=ident[:DIM, :DIM],
                    )
                ot = finp.tile([P, GB * DIM], f32, tag="ot")
                nc.vector.tensor_tensor(
                    out=ot[:, : blks * DIM],
                    in0=po[:, : blks * DIM],
                    in1=xs_t[:, g * GB * DIM : (g * GB + blks) * DIM],
                    op=add,
                )
                row0 = g * GB * P
                rows = min(SHARD - row0, blks * P)
                fb2 = rows // P
                if fb2:
                    nc.sync.dma_start(
                        out=out_d[row0 : row0 + fb2 * P, :].rearrange(
                            "(bb p) d -> p bb d", p=P
                        ),
                        in_=ot[:, : fb2 * DIM].rearrange("p (bb d) -> p bb d", d=DIM),
                    )
                rem2 = rows - fb2 * P
                if rem2:
                    nc.sync.dma_start(
                        out=out_d[row0 + fb2 * P : row0 + rows, :],
                        in_=ot[:rem2, fb2 * DIM : (fb2 + 1) * DIM],
                    )

    nc.compile()
    return nc


def run(plan, nc, in_maps, trace=False, tmpdir=None):
    _install_ntff_hook()
    from concourse.bass_utils import run_bass_kernel_spmd

    res = run_bass_kernel_spmd(
        nc,
        in_maps,
        core_ids=list(range(plan.n_cores)),
        trace=trace,
        tmpdir=tmpdir,
    )
    outs = [res.results[i]["out"] for i in range(plan.n_cores)]
    return np.concatenate(outs, axis=0), res


_CACHE = {}


def kernel(x, edge_index, W, b):
    plan, in_maps = preprocess(x, edge_index, W, b)
    sig = tuple(tuple(s) for s in plan.pass_sizes)
    ent = _CACHE.get("prog")
    if ent is None or ent[0] != sig:
        nc = build_program(plan)
        _CACHE["prog"] = (sig, nc)
    nc = _CACHE["prog"][1]
    out, _ = run(plan, nc, in_maps)
    return out
